# revision 15
# baseline (speedup 1.0000x reference)
"""Bass/Tile kernel for masked multi-head attention on 8 trn2 NeuronCores.

Problem (hardcoded shapes): B=4, S=2048, DM=1024, H=16, D=64.
  q = Q_seq @ WQ, k = K_seq @ WK, v = V_seq @ WV  (per-head split, D=64)
  A = softmax(q k^T / 8  masked to keys < V_len[b])
  O = (A v) masked to queries < Q_len[b]

Sharding: core c owns head pair hp=c (heads 2c, 2c+1) of EVERY batch.
All cores run an identical (SPMD) program; per-core data = W column slices.
This balances attention, projection and DMA work exactly 8 ways.

Device layout:
  - Host pre-transposes Q/K/V to DM-major [1024, W] (W = per-batch width
    rounded to 64 for queries / 128 for keys, zero-padded) so projections
    contract over partitions with clean DMAs. 1/sqrt(D) folded into WQ.
  - Scores are computed TRANSPOSED (keys on partitions, queries on the
    free dim) so the AV matmul consumes exp(scores) directly; no
    max-subtraction needed (scores are O(1) here).
  - The softmax denominator is folded into the AV matmul as a 65th
    "ones" column of the projected V tile (masked per-key at the V_len
    boundary), so each attention cell is just 4 matmuls:
    2 packed score matmuls + 2 AV matmuls of width 65.
  - Matmul free dims are trimmed to the actual query width per batch.
  - fp16 on-chip intermediates (more mantissa than bf16 at equal speed),
    fp32 PSUM. Output is unnormalized O^T plus denominators; the host
    divides, transposes, applies the query mask and assembles.
"""

import math
import os

import ml_dtypes
import numpy as np

B, S, DM, H, D = 4, 2048, 1024, 16, 64
P = 128
NCORES = 8
SPAN = 512  # max matmul free dim (one PSUM bank of fp32)

LAST_EXEC_NS = None
LAST_RESULTS = None
LAST_NC = None
LAST_IN_MAPS = None

_PROGRAM_CACHE = {}


def _ceil(a, b):
    return -(-a // b)


def _splits(total, chunk=SPAN):
    """[(offset, width), ...] covering `total` in chunks of ≤`chunk`."""
    out = []
    off = 0
    while off < total:
        w = min(chunk, total - off)
        out.append((off, w))
        off += w
    return out


def _split_excess_waits(nc, mybir):
    """Move semaphore waits beyond each instruction's encoding limit onto
    preceding same-engine NoOps.  This walrus build rejects any op carrying
    more than one sync wait ("Too many sync wait commands"), but an
    engine-level NoOp can hold the wait instead — the engine stalls on the
    NoOp, then issues the real instruction."""
    uid = 0
    for fn in nc.m.functions:
        for blk in fn.blocks:
            insts = blk.instructions
            out = []
            changed = False
            for inst in insts:
                si = inst.sync_info
                waits = list(si.on_wait) if si is not None and si.on_wait else []
                limit = int(os.environ.get("WAIT_LIMIT", "1"))
                if len(waits) > limit:
                    for w in waits[:-limit] if limit else waits:
                        nop = mybir.InstNoOp(name=f"wsplit-{uid}", ins=[],
                                             outs=[])
                        uid += 1
                        nop.engine = inst.engine
                        nop.sync_info = mybir.SyncInfo(on_wait=[w],
                                                       on_update=[])
                        out.append(nop)
                    si.on_wait = waits[-limit:] if limit else []
                    changed = True
                out.append(inst)
            if changed:
                blk.instructions = out


def _widths(qlen, vlen):
    """Per-batch padded query width (64-aligned) and key width
    (128-aligned), zero for inactive batches."""
    qw, kw = [], []
    for b in range(B):
        if qlen[b] > 0 and vlen[b] > 0:
            qw.append(_ceil(qlen[b], 64) * 64)
            kw.append(_ceil(vlen[b], P) * P)
        else:
            qw.append(0)
            kw.append(0)
    return qw, kw


def _build_program(qlen, vlen):
    """Build the SPMD Bass program for the given per-batch lengths."""
    import concourse.bass as bass
    import concourse.mybir as mybir
    import concourse.tile as tile

    f16 = mybir.dt.float16
    f32 = mybir.dt.float32
    AF = mybir.ActivationFunctionType

    qw, kw = _widths(qlen, vlen)
    # Batch order: smallest staging first (fast PE ramp), then the largest
    # batch (its ACT-bound attention overlaps later batches' projections).
    active = [b for b in range(B) if qw[b] > 0]
    order = os.environ.get("BATCH_ORDER", "")
    if order:
        perm = [int(x) for x in order.split(",")]
        active = [b for b in perm if b in active]
    else:
        # Descending attention area: the long ACT-bound attention of big
        # batches overlaps later (PE-dense) projection phases, and the
        # kernel tail ends on a small, quickly-drained batch.
        active.sort(key=lambda b: -(kw[b] * qw[b]))

    nc = bass.Bass(
        "TRN2",
        target_bir_lowering=False,
        debug=False,
        enable_asserts=False,
        num_devices=NCORES,
    )

    qt_d, kt_d, vt_d, ot_d = {}, {}, {}, {}
    for b in active:
        qt_d[b] = nc.dram_tensor(f"qt{b}", [DM, qw[b]], f16,
                                 kind="ExternalInput").ap()
        kt_d[b] = nc.dram_tensor(f"kt{b}", [DM, kw[b]], f16,
                                 kind="ExternalInput").ap()
        vt_d[b] = nc.dram_tensor(f"vt{b}", [DM, kw[b]], f16,
                                 kind="ExternalInput").ap()
        ot_d[b] = nc.dram_tensor(f"ot{b}", [65, 2, qw[b]], f32,
                                 kind="ExternalOutput").ap()
    wq_d = nc.dram_tensor("wq", [DM, P], f16, kind="ExternalInput").ap()
    wk_d = nc.dram_tensor("wk", [DM, P], f16, kind="ExternalInput").ap()
    wv_d = nc.dram_tensor("wv", [DM, P], f16, kind="ExternalInput").ap()

    NCH = DM // P  # contraction chunks per projection

    with tile.TileContext(nc) as tc:
        with (
            tc.tile_pool(name="wpool", bufs=1) as wpool,
            tc.tile_pool(name="proj", bufs=2) as projpool,
            tc.tile_pool(name="stage", bufs=8) as stage,
            tc.tile_pool(name="sbig", bufs=3) as sbig,
            tc.tile_pool(name="outp", bufs=2) as outp,
            # PSUM budget (8 banks): pproj 2×1 + psc 2×2 + po0/po1 1×1 each
            tc.tile_pool(name="pproj", bufs=2, space="PSUM") as pproj,
            tc.tile_pool(name="psc", bufs=2, space="PSUM") as pscp,
            tc.tile_pool(name="ppo", bufs=1, space="PSUM") as ppo,
        ):
            # Weights for this core's head pair, DM on partitions: [128,8,128]
            # wq is loaded first; wk/wv are emitted after the first staging
            # DMA below so the first projection isn't queued behind them.
            w_sb = {}
            for name, ap in (("wq", wq_d), ("wk", wk_d), ("wv", wv_d)):
                w_sb[name] = wpool.tile([P, NCH, P], f16, tag=f"w_{name}",
                                        name=f"w_{name}")
            nc.sync.dma_start(w_sb["wq"],
                              wq_d.rearrange("(c p) m -> p c m", p=P))
            first_stage = [True]

            def _stage(src_ap, w):
                st = stage.tile([P, NCH, w], f16, tag="stage")
                nc.sync.dma_start(st,
                                  src_ap.rearrange("(c p) n -> p c n", p=P))
                if first_stage[0]:
                    first_stage[0] = False
                    nc.sync.dma_start(
                        w_sb["wk"], wk_d.rearrange("(c p) m -> p c m", p=P))
                    nc.sync.dma_start(
                        w_sb["wv"], wv_d.rearrange("(c p) m -> p c m", p=P))
                return st

            for b in active:
                nkt = kw[b] // P

                qT = projpool.tile([P, qw[b]], f16, tag="qT")
                kT = projpool.tile([P, kw[b]], f16, tag="kT")
                # projected V + ones column per head: [kpos, head, ktile, 65]
                vnx = projpool.tile([P, 2, nkt, 65], f16, tag="vnx")

                # --- projections (per ≤512-wide span, double-buffered DMA)
                for off, w in _splits(qw[b]):
                    st = _stage(qt_d[b][:, off:off + w], w)
                    ps = pproj.tile([P, SPAN], f32, tag="pproj")
                    for ch in range(NCH):
                        nc.tensor.matmul(ps[:, :w], lhsT=w_sb["wq"][:, ch],
                                         rhs=st[:, ch],
                                         start=(ch == 0),
                                         stop=(ch == NCH - 1))
                    nc.vector.tensor_copy(qT[:, off:off + w], ps[:, :w])
                for off, w in _splits(kw[b]):
                    st = _stage(kt_d[b][:, off:off + w], w)
                    ps = pproj.tile([P, SPAN], f32, tag="pproj")
                    for ch in range(NCH):
                        nc.tensor.matmul(ps[:, :w], lhsT=w_sb["wk"][:, ch],
                                         rhs=st[:, ch],
                                         start=(ch == 0),
                                         stop=(ch == NCH - 1))
                    nc.vector.tensor_copy(kT[:, off:off + w], ps[:, :w])
                for off, w in _splits(kw[b]):
                    st = _stage(vt_d[b][:, off:off + w], w)
                    for kt in range(off // P, (off + w) // P):
                        o = kt * P - off
                        pv = pproj.tile([P, 2, 64], f32, tag="pproj")
                        for ch in range(NCH):
                            nc.tensor.matmul(pv, lhsT=st[:, ch, o:o + P],
                                             rhs=w_sb["wv"][:, ch],
                                             start=(ch == 0),
                                             stop=(ch == NCH - 1))
                        nc.vector.tensor_copy(vnx[:, :, kt, 0:64], pv)

                # ones columns (masked at the V_len boundary tile)
                nfull = vlen[b] // P
                if nfull > 0:
                    nc.vector.memset(vnx[:, :, 0:nfull, 64:65], 1.0)
                if nfull < nkt:  # partial boundary tile
                    r = vlen[b] - nfull * P
                    nc.vector.memset(vnx[:, :, nfull:nkt, 64:65], 0.0)
                    nc.vector.memset(vnx[0:r, :, nfull:nkt, 64:65], 1.0)

                # --- attention
                for off, w in _splits(qw[b]):
                    po0 = ppo.tile([65, SPAN], f32, tag="po0")
                    po1 = ppo.tile([65, SPAN], f32, tag="po1")
                    for kt in range(nkt):
                        ks = slice(kt * P, (kt + 1) * P)
                        psc = pscp.tile([P, 2, SPAN], f32, tag="psc")
                        nc.tensor.matmul(psc[:, 0, :w],
                                         lhsT=kT[0:64, ks],
                                         rhs=qT[0:64, off:off + w],
                                         start=True, stop=True,
                                         tile_position=(0, 0))
                        nc.tensor.matmul(psc[:, 1, :w],
                                         lhsT=kT[64:P, ks],
                                         rhs=qT[64:P, off:off + w],
                                         start=True, stop=True,
                                         tile_position=(64, 0))
                        ex = sbig.tile([P, 2, SPAN], f16, tag="exp")
                        nc.scalar.activation(ex[:, :, :w], psc[:, :, :w],
                                             AF.Exp)
                        first, last = (kt == 0), (kt == nkt - 1)
                        nc.tensor.matmul(po0[:, :w], lhsT=vnx[:, 0, kt],
                                         rhs=ex[:, 0, :w],
                                         start=first, stop=last)
                        nc.tensor.matmul(po1[:, :w], lhsT=vnx[:, 1, kt],
                                         rhs=ex[:, 1, :w],
                                         start=first, stop=last)
                    osb = outp.tile([65, 2, SPAN], f32, tag="osb")
                    nc.vector.tensor_copy(osb[:, 0, :w], po0[:, :w])
                    nc.vector.tensor_copy(osb[:, 1, :w], po1[:, :w])
                    nc.sync.dma_start(ot_d[b][:, :, off:off + w],
                                      osb[:, :, :w])

    _split_excess_waits(nc, mybir)
    return nc, qw, kw, active


def kernel(Q_seq, K_seq, V_seq, Q_len, V_len, WQ, WK, WV):
    global LAST_EXEC_NS, LAST_RESULTS, LAST_NC, LAST_IN_MAPS
    import concourse.bass_utils as bass_utils

    Q_seq = np.ascontiguousarray(np.asarray(Q_seq, dtype=np.float32))
    K_seq = np.ascontiguousarray(np.asarray(K_seq, dtype=np.float32))
    V_seq = np.ascontiguousarray(np.asarray(V_seq, dtype=np.float32))
    WQ = np.asarray(WQ, dtype=np.float32)
    WK = np.asarray(WK, dtype=np.float32)
    WV = np.asarray(WV, dtype=np.float32)
    qlen = [int(x) for x in np.asarray(Q_len).ravel()]
    vlen = [int(x) for x in np.asarray(V_len).ravel()]

    f16 = ml_dtypes.float16 if hasattr(ml_dtypes, "float16") else np.float16
    out = np.zeros((B, S, H * D), dtype=np.float32)

    # Degenerate batches (V_len==0): reference softmax of an all-masked row
    # is uniform over all S keys -> O row = mean of v rows.
    for b in range(B):
        if vlen[b] == 0 and qlen[b] > 0:
            v = V_seq[b] @ WV
            out[b, :qlen[b], :] = v.mean(axis=0, keepdims=True)

    key = (tuple(qlen), tuple(vlen))
    if key not in _PROGRAM_CACHE:
        _PROGRAM_CACHE[key] = _build_program(qlen, vlen)
    nc, qw, kw, active = _PROGRAM_CACHE[key]

    if active:
        WQs = (WQ / math.sqrt(D)).astype(f16)
        WKs = WK.astype(f16)
        WVs = WV.astype(f16)

        # Shared (core-independent) transposed activations, zero-padded.
        shared = {}
        for b in active:
            qt = np.zeros((DM, qw[b]), dtype=f16)
            qt[:, :qlen[b]] = Q_seq[b, :qlen[b], :].T
            kt = np.zeros((DM, kw[b]), dtype=f16)
            kt[:, :vlen[b]] = K_seq[b, :vlen[b], :].T
            vt = np.zeros((DM, kw[b]), dtype=f16)
            vt[:, :vlen[b]] = V_seq[b, :vlen[b], :].T
            shared[f"qt{b}"] = qt
            shared[f"kt{b}"] = kt
            shared[f"vt{b}"] = vt

        in_maps = []
        for c in range(NCORES):
            m = dict(shared)
            sl = slice(c * P, (c + 1) * P)
            m["wq"] = np.ascontiguousarray(WQs[:, sl])
            m["wk"] = np.ascontiguousarray(WKs[:, sl])
            m["wv"] = np.ascontiguousarray(WVs[:, sl])
            in_maps.append(m)

        trace = bool(int(os.environ.get("KERNEL_TRACE", "0")))
        try:
            res = bass_utils.run_bass_kernel_spmd(
                nc, in_maps, core_ids=list(range(NCORES)), trace=trace)
        except ModuleNotFoundError:
            # Profiling hook unavailable in this container; run untraced.
            os.environ["BASS_NEVER_TRACE"] = "1"
            res = bass_utils.run_bass_kernel_spmd(
                nc, in_maps, core_ids=list(range(NCORES)), trace=False)
        LAST_EXEC_NS = res.exec_time_ns
        LAST_RESULTS = res
        LAST_NC = nc
        LAST_IN_MAPS = in_maps

        for c in range(NCORES):
            r = res.results[c]
            for b in active:
                arr = r[f"ot{b}"]  # [65, 2, qw]: rows 0-63 O^T, row 64 den
                n = qlen[b]
                for h in (0, 1):
                    head = 2 * c + h
                    num = arr[0:64, h, :n]
                    den = arr[64, h, :n]
                    out[b, :n, head * 64:(head + 1) * 64] = \
                        (num / den[None, :]).T
    return out


# revision 24
# speedup vs baseline: 1.0105x; 1.0105x over previous
"""Bass/Tile kernel for masked multi-head attention on 8 trn2 NeuronCores.

Problem (hardcoded shapes): B=4, S=2048, DM=1024, H=16, D=64.
  q = Q_seq @ WQ, k = K_seq @ WK, v = V_seq @ WV  (per-head split, D=64)
  A = softmax(q k^T / 8  masked to keys < V_len[b])
  O = (A v) masked to queries < Q_len[b]

Sharding: core c owns head pair hp=c (heads 2c, 2c+1) of EVERY batch.
All cores run an identical (SPMD) program; per-core data = W column slices.
This balances attention, projection and DMA work exactly 8 ways.

Device layout:
  - Host pre-transposes Q/K/V to DM-major [1024, W] (W = per-batch width
    rounded to 64 for queries / 128 for keys, zero-padded) so projections
    contract over partitions with clean DMAs. 1/sqrt(D) folded into WQ.
  - Scores are computed TRANSPOSED (keys on partitions, queries on the
    free dim) so the AV matmul consumes exp(scores) directly; no
    max-subtraction needed (scores are O(1) here).
  - The softmax denominator is folded into the AV matmul as a 65th
    "ones" column of the projected V tile (masked per-key at the V_len
    boundary), so each attention cell is just 4 matmuls:
    2 packed score matmuls + 2 AV matmuls of width 65.
  - Matmul free dims are trimmed to the actual query width per batch.
  - fp16 on-chip intermediates (more mantissa than bf16 at equal speed),
    fp32 PSUM. Output is unnormalized O^T plus denominators; the host
    divides, transposes, applies the query mask and assembles.
"""

import math
import os

import ml_dtypes
import numpy as np

B, S, DM, H, D = 4, 2048, 1024, 16, 64
P = 128
NCORES = 8
SPAN = 512  # max matmul free dim (one PSUM bank of fp32)

LAST_EXEC_NS = None
LAST_RESULTS = None
LAST_NC = None
LAST_IN_MAPS = None

_PROGRAM_CACHE = {}


def _ceil(a, b):
    return -(-a // b)


def _splits(total, chunk=SPAN):
    """[(offset, width), ...] covering `total` in chunks of ≤`chunk`."""
    out = []
    off = 0
    while off < total:
        w = min(chunk, total - off)
        out.append((off, w))
        off += w
    return out


def _split_excess_waits(nc, mybir):
    """Move semaphore waits beyond each instruction's encoding limit onto
    preceding same-engine NoOps.  This walrus build rejects any op carrying
    more than one sync wait ("Too many sync wait commands"), but an
    engine-level NoOp can hold the wait instead — the engine stalls on the
    NoOp, then issues the real instruction."""
    uid = 0
    for fn in nc.m.functions:
        for blk in fn.blocks:
            insts = blk.instructions
            out = []
            changed = False
            for inst in insts:
                si = inst.sync_info
                waits = list(si.on_wait) if si is not None and si.on_wait else []
                limit = int(os.environ.get("WAIT_LIMIT", "1"))
                if len(waits) > limit:
                    for w in waits[:-limit] if limit else waits:
                        nop = mybir.InstNoOp(name=f"wsplit-{uid}", ins=[],
                                             outs=[])
                        uid += 1
                        nop.engine = inst.engine
                        nop.sync_info = mybir.SyncInfo(on_wait=[w],
                                                       on_update=[])
                        out.append(nop)
                    si.on_wait = waits[-limit:] if limit else []
                    changed = True
                out.append(inst)
            if changed:
                blk.instructions = out


def _widths(qlen, vlen):
    """Per-batch padded query width (64-aligned) and key width
    (128-aligned), zero for inactive batches."""
    qw, kw = [], []
    for b in range(B):
        if qlen[b] > 0 and vlen[b] > 0:
            qw.append(_ceil(qlen[b], 64) * 64)
            kw.append(_ceil(vlen[b], P) * P)
        else:
            qw.append(0)
            kw.append(0)
    return qw, kw


def _build_program(qlen, vlen):
    """Build the SPMD Bass program for the given per-batch lengths."""
    import concourse.bass as bass
    import concourse.mybir as mybir
    import concourse.tile as tile

    f16 = mybir.dt.float16
    f32 = mybir.dt.float32
    AF = mybir.ActivationFunctionType

    qw, kw = _widths(qlen, vlen)
    # Batch order: smallest staging first (fast PE ramp), then the largest
    # batch (its ACT-bound attention overlaps later batches' projections).
    active = [b for b in range(B) if qw[b] > 0]
    order = os.environ.get("BATCH_ORDER", "")
    if order:
        perm = [int(x) for x in order.split(",")]
        active = [b for b in perm if b in active]
    else:
        # Descending attention area: the long ACT-bound attention of big
        # batches overlaps later (PE-dense) projection phases, and the
        # kernel tail ends on a small, quickly-drained batch.
        active.sort(key=lambda b: -(kw[b] * qw[b]))

    nc = bass.Bass(
        "TRN2",
        target_bir_lowering=False,
        debug=False,
        enable_asserts=False,
        num_devices=NCORES,
    )

    qt_d, kt_d, vt_d, ot_d = {}, {}, {}, {}
    for b in active:
        qt_d[b] = nc.dram_tensor(f"qt{b}", [DM, qw[b]], f16,
                                 kind="ExternalInput").ap()
        kt_d[b] = nc.dram_tensor(f"kt{b}", [DM, kw[b]], f16,
                                 kind="ExternalInput").ap()
        vt_d[b] = nc.dram_tensor(f"vt{b}", [DM, kw[b]], f16,
                                 kind="ExternalInput").ap()
        ot_d[b] = nc.dram_tensor(f"ot{b}", [65, 2, qw[b]], f32,
                                 kind="ExternalOutput").ap()
    wq_d = nc.dram_tensor("wq", [DM, P], f16, kind="ExternalInput").ap()
    wk_d = nc.dram_tensor("wk", [DM, P], f16, kind="ExternalInput").ap()
    wv_d = nc.dram_tensor("wv", [DM, P], f16, kind="ExternalInput").ap()

    NCH = DM // P  # contraction chunks per projection

    with tile.TileContext(nc) as tc:
        with (
            tc.tile_pool(name="wpool", bufs=1) as wpool,
            tc.tile_pool(name="proj", bufs=3) as projpool,
            tc.tile_pool(name="stage", bufs=12) as stage,
            tc.tile_pool(name="sbig", bufs=4) as sbig,
            tc.tile_pool(name="outp", bufs=2) as outp,
            # PSUM budget (8 banks): pproj 2×1 + psc 2×2 + po0/po1 1×1 each
            tc.tile_pool(name="pproj", bufs=2, space="PSUM") as pproj,
            tc.tile_pool(name="psc", bufs=2, space="PSUM") as pscp,
            tc.tile_pool(name="ppo", bufs=1, space="PSUM") as ppo,
        ):
            # Weights for this core's head pair, DM on partitions: [128,8,128]
            # wq is loaded first; wk/wv are emitted after the first staging
            # DMA below so the first projection isn't queued behind them.
            w_sb = {}
            for name, ap in (("wq", wq_d), ("wk", wk_d), ("wv", wv_d)):
                w_sb[name] = wpool.tile([P, NCH, P], f16, tag=f"w_{name}",
                                        name=f"w_{name}")
            nc.sync.dma_start(w_sb["wq"],
                              wq_d.rearrange("(c p) m -> p c m", p=P))

            # Warm the PE HAM clock gate during the unavoidable initial DMA
            # wait: ~4us of dummy matmuls flips the PE from 1.2 to 2.4 GHz
            # before the first real projection arrives.
            if int(os.environ.get("PE_WARMUP", "1")):
                warm = wpool.tile([P, P], f16, tag="warm")
                nc.vector.memset(warm, 0.0)
                pwm = pproj.tile([P, 64], f32, tag="pproj")
                nwarm = 80
                for i in range(nwarm):
                    nc.tensor.matmul(pwm, lhsT=warm, rhs=warm[:, 0:64],
                                     start=(i == 0), stop=(i == nwarm - 1))
            first_stage = [True]

            def _stage(src_ap, w):
                st = stage.tile([P, NCH, w], f16, tag="stage")
                nc.sync.dma_start(st,
                                  src_ap.rearrange("(c p) n -> p c n", p=P))
                if first_stage[0]:
                    first_stage[0] = False
                    nc.sync.dma_start(
                        w_sb["wk"], wk_d.rearrange("(c p) m -> p c m", p=P))
                    nc.sync.dma_start(
                        w_sb["wv"], wv_d.rearrange("(c p) m -> p c m", p=P))
                return st

            for bi, b in enumerate(active):
                nkt = kw[b] // P

                qT = projpool.tile([P, qw[b]], f16, tag="qT")
                kT = projpool.tile([P, kw[b]], f16, tag="kT")
                # projected V + ones column per head: [kpos, head, ktile, 65]
                vnx = projpool.tile([P, 2, nkt, 65], f16, tag="vnx")

                # --- projections, q/k/v spans interleaved so the first
                # attention cells' inputs land as early as possible
                def _proj_q(off, w):
                    st = _stage(qt_d[b][:, off:off + w], w)
                    ps = pproj.tile([P, SPAN], f32, tag="pproj",
                                    name="ps_q")
                    for ch in range(NCH):
                        nc.tensor.matmul(ps[:, :w], lhsT=w_sb["wq"][:, ch],
                                         rhs=st[:, ch],
                                         start=(ch == 0),
                                         stop=(ch == NCH - 1))
                    nc.vector.tensor_copy(qT[:, off:off + w], ps[:, :w])

                def _proj_k(off, w):
                    st = _stage(kt_d[b][:, off:off + w], w)
                    ps = pproj.tile([P, SPAN], f32, tag="pproj",
                                    name="ps_k")
                    for ch in range(NCH):
                        nc.tensor.matmul(ps[:, :w], lhsT=w_sb["wk"][:, ch],
                                         rhs=st[:, ch],
                                         start=(ch == 0),
                                         stop=(ch == NCH - 1))
                    nc.vector.tensor_copy(kT[:, off:off + w], ps[:, :w])

                def _proj_v(off, w):
                    st = _stage(vt_d[b][:, off:off + w], w)
                    for kt in range(off // P, (off + w) // P):
                        o = kt * P - off
                        pv = pproj.tile([P, 2, 64], f32, tag="pproj",
                                        name="pv")
                        for ch in range(NCH):
                            nc.tensor.matmul(pv, lhsT=st[:, ch, o:o + P],
                                             rhs=w_sb["wv"][:, ch],
                                             start=(ch == 0),
                                             stop=(ch == NCH - 1))
                        nc.vector.tensor_copy(vnx[:, :, kt, 0:64], pv)

                steps = ([("q", s) for s in _splits(qw[b])] +
                         [("k", s) for s in _splits(kw[b])] +
                         [("v", s) for s in _splits(kw[b])])
                if os.environ.get("PROJ_INTERLEAVE", "1") == "1":
                    qs = [("q", s) for s in _splits(qw[b])]
                    ks = [("k", s) for s in _splits(kw[b])]
                    vs = [("v", s) for s in _splits(kw[b])]
                    steps = []
                    n = max(len(qs), len(ks), len(vs))
                    for i in range(n):
                        for lst in (qs, ks, vs):
                            if i < len(lst):
                                steps.append(lst[i])
                for kind, (off, w) in steps:
                    {"q": _proj_q, "k": _proj_k, "v": _proj_v}[kind](off, w)

                # ones columns (masked at the V_len boundary tile)
                nfull = vlen[b] // P
                if nfull > 0:
                    nc.vector.memset(vnx[:, :, 0:nfull, 64:65], 1.0)
                if nfull < nkt:  # partial boundary tile
                    r = vlen[b] - nfull * P
                    nc.vector.memset(vnx[:, :, nfull:nkt, 64:65], 0.0)
                    nc.vector.memset(vnx[0:r, :, nfull:nkt, 64:65], 1.0)

                # --- attention
                for off, w in _splits(qw[b]):
                    po0 = ppo.tile([65, SPAN], f32, tag="po0")
                    po1 = ppo.tile([65, SPAN], f32, tag="po1")
                    for kt in range(nkt):
                        ks = slice(kt * P, (kt + 1) * P)
                        psc = pscp.tile([P, 2, SPAN], f32, tag="psc")
                        nc.tensor.matmul(psc[:, 0, :w],
                                         lhsT=kT[0:64, ks],
                                         rhs=qT[0:64, off:off + w],
                                         start=True, stop=True,
                                         tile_position=(0, 0))
                        nc.tensor.matmul(psc[:, 1, :w],
                                         lhsT=kT[64:P, ks],
                                         rhs=qT[64:P, off:off + w],
                                         start=True, stop=True,
                                         tile_position=(64, 0))
                        ex = sbig.tile([P, 2, SPAN], f16, tag="exp")
                        nc.scalar.activation(ex[:, :, :w], psc[:, :, :w],
                                             AF.Exp)
                        first, last = (kt == 0), (kt == nkt - 1)
                        nc.tensor.matmul(po0[:, :w], lhsT=vnx[:, 0, kt],
                                         rhs=ex[:, 0, :w],
                                         start=first, stop=last)
                        nc.tensor.matmul(po1[:, :w], lhsT=vnx[:, 1, kt],
                                         rhs=ex[:, 1, :w],
                                         start=first, stop=last)
                    osb = outp.tile([65, 2, SPAN], f32, tag="osb")
                    nc.vector.tensor_copy(osb[:, 0, :w], po0[:, :w])
                    nc.vector.tensor_copy(osb[:, 1, :w], po1[:, :w])
                    nc.sync.dma_start(ot_d[b][:, :, off:off + w],
                                      osb[:, :, :w])

    _split_excess_waits(nc, mybir)
    return nc, qw, kw, active


def kernel(Q_seq, K_seq, V_seq, Q_len, V_len, WQ, WK, WV):
    global LAST_EXEC_NS, LAST_RESULTS, LAST_NC, LAST_IN_MAPS
    import concourse.bass_utils as bass_utils

    Q_seq = np.ascontiguousarray(np.asarray(Q_seq, dtype=np.float32))
    K_seq = np.ascontiguousarray(np.asarray(K_seq, dtype=np.float32))
    V_seq = np.ascontiguousarray(np.asarray(V_seq, dtype=np.float32))
    WQ = np.asarray(WQ, dtype=np.float32)
    WK = np.asarray(WK, dtype=np.float32)
    WV = np.asarray(WV, dtype=np.float32)
    qlen = [int(x) for x in np.asarray(Q_len).ravel()]
    vlen = [int(x) for x in np.asarray(V_len).ravel()]

    f16 = ml_dtypes.float16 if hasattr(ml_dtypes, "float16") else np.float16
    out = np.zeros((B, S, H * D), dtype=np.float32)

    # Degenerate batches (V_len==0): reference softmax of an all-masked row
    # is uniform over all S keys -> O row = mean of v rows.
    for b in range(B):
        if vlen[b] == 0 and qlen[b] > 0:
            v = V_seq[b] @ WV
            out[b, :qlen[b], :] = v.mean(axis=0, keepdims=True)

    key = (tuple(qlen), tuple(vlen))
    if key not in _PROGRAM_CACHE:
        _PROGRAM_CACHE[key] = _build_program(qlen, vlen)
    nc, qw, kw, active = _PROGRAM_CACHE[key]

    if active:
        WQs = (WQ / math.sqrt(D)).astype(f16)
        WKs = WK.astype(f16)
        WVs = WV.astype(f16)

        # Shared (core-independent) transposed activations, zero-padded.
        shared = {}
        for b in active:
            qt = np.zeros((DM, qw[b]), dtype=f16)
            qt[:, :qlen[b]] = Q_seq[b, :qlen[b], :].T
            kt = np.zeros((DM, kw[b]), dtype=f16)
            kt[:, :vlen[b]] = K_seq[b, :vlen[b], :].T
            vt = np.zeros((DM, kw[b]), dtype=f16)
            vt[:, :vlen[b]] = V_seq[b, :vlen[b], :].T
            shared[f"qt{b}"] = qt
            shared[f"kt{b}"] = kt
            shared[f"vt{b}"] = vt

        in_maps = []
        for c in range(NCORES):
            m = dict(shared)
            sl = slice(c * P, (c + 1) * P)
            m["wq"] = np.ascontiguousarray(WQs[:, sl])
            m["wk"] = np.ascontiguousarray(WKs[:, sl])
            m["wv"] = np.ascontiguousarray(WVs[:, sl])
            in_maps.append(m)

        trace = bool(int(os.environ.get("KERNEL_TRACE", "0")))
        try:
            res = bass_utils.run_bass_kernel_spmd(
                nc, in_maps, core_ids=list(range(NCORES)), trace=trace)
        except ModuleNotFoundError:
            # Profiling hook unavailable in this container; run untraced.
            os.environ["BASS_NEVER_TRACE"] = "1"
            res = bass_utils.run_bass_kernel_spmd(
                nc, in_maps, core_ids=list(range(NCORES)), trace=False)
        LAST_EXEC_NS = res.exec_time_ns
        LAST_RESULTS = res
        LAST_NC = nc
        LAST_IN_MAPS = in_maps

        for c in range(NCORES):
            r = res.results[c]
            for b in active:
                arr = r[f"ot{b}"]  # [65, 2, qw]: rows 0-63 O^T, row 64 den
                n = qlen[b]
                for h in (0, 1):
                    head = 2 * c + h
                    num = arr[0:64, h, :n]
                    den = arr[64, h, :n]
                    out[b, :n, head * 64:(head + 1) * 64] = \
                        (num / den[None, :]).T
    return out


# revision 27
# speedup vs baseline: 1.0331x; 1.0224x over previous
"""Bass/Tile kernel for masked multi-head attention on 8 trn2 NeuronCores.

Problem (hardcoded shapes): B=4, S=2048, DM=1024, H=16, D=64.
  q = Q_seq @ WQ, k = K_seq @ WK, v = V_seq @ WV  (per-head split, D=64)
  A = softmax(q k^T / 8  masked to keys < V_len[b])
  O = (A v) masked to queries < Q_len[b]

Sharding: core c owns head pair hp=c (heads 2c, 2c+1) of EVERY batch.
All cores run an identical (SPMD) program; per-core data = W column slices.
This balances attention, projection and DMA work exactly 8 ways.

Device layout:
  - Host pre-transposes Q/K/V to DM-major [1024, W] (W = per-batch width
    rounded to 64 for queries / 128 for keys, zero-padded) so projections
    contract over partitions with clean DMAs. 1/sqrt(D) folded into WQ.
  - Scores are computed TRANSPOSED (keys on partitions, queries on the
    free dim) so the AV matmul consumes exp(scores) directly; no
    max-subtraction needed (scores are O(1) here).
  - The softmax denominator is folded into the AV matmul as a 65th
    "ones" column of the projected V tile (masked per-key at the V_len
    boundary), so each attention cell is just 4 matmuls:
    2 packed score matmuls + 2 AV matmuls of width 65.
  - Matmul free dims are trimmed to the actual query width per batch.
  - fp16 on-chip intermediates (more mantissa than bf16 at equal speed),
    fp32 PSUM. Output is unnormalized O^T plus denominators; the host
    divides, transposes, applies the query mask and assembles.
"""

import math
import os

import ml_dtypes
import numpy as np

B, S, DM, H, D = 4, 2048, 1024, 16, 64
P = 128
NCORES = 8
SPAN = 512  # max matmul free dim (one PSUM bank of fp32)

LAST_EXEC_NS = None
LAST_RESULTS = None
LAST_NC = None
LAST_IN_MAPS = None

_PROGRAM_CACHE = {}


def _ceil(a, b):
    return -(-a // b)


def _splits(total, chunk=SPAN):
    """[(offset, width), ...] covering `total` in chunks of ≤`chunk`."""
    out = []
    off = 0
    while off < total:
        w = min(chunk, total - off)
        out.append((off, w))
        off += w
    return out


def _split_excess_waits(nc, mybir):
    """Move semaphore waits beyond each instruction's encoding limit onto
    preceding same-engine NoOps.  This walrus build rejects any op carrying
    more than one sync wait ("Too many sync wait commands"), but an
    engine-level NoOp can hold the wait instead — the engine stalls on the
    NoOp, then issues the real instruction."""
    uid = 0
    for fn in nc.m.functions:
        for blk in fn.blocks:
            insts = blk.instructions
            out = []
            changed = False
            for inst in insts:
                si = inst.sync_info
                waits = list(si.on_wait) if si is not None and si.on_wait else []
                limit = int(os.environ.get("WAIT_LIMIT", "1"))
                if len(waits) > limit:
                    for w in waits[:-limit] if limit else waits:
                        nop = mybir.InstNoOp(name=f"wsplit-{uid}", ins=[],
                                             outs=[])
                        uid += 1
                        nop.engine = inst.engine
                        nop.sync_info = mybir.SyncInfo(on_wait=[w],
                                                       on_update=[])
                        out.append(nop)
                    si.on_wait = waits[-limit:] if limit else []
                    changed = True
                out.append(inst)
            if changed:
                blk.instructions = out


def _widths(qlen, vlen):
    """Per-batch padded query width (64-aligned) and key width
    (128-aligned), zero for inactive batches."""
    qw, kw = [], []
    for b in range(B):
        if qlen[b] > 0 and vlen[b] > 0:
            qw.append(_ceil(qlen[b], 64) * 64)
            kw.append(_ceil(vlen[b], P) * P)
        else:
            qw.append(0)
            kw.append(0)
    return qw, kw


def _build_program(qlen, vlen):
    """Build the SPMD Bass program for the given per-batch lengths."""
    import concourse.bass as bass
    import concourse.mybir as mybir
    import concourse.tile as tile

    f16 = mybir.dt.float16
    f32 = mybir.dt.float32
    AF = mybir.ActivationFunctionType

    qw, kw = _widths(qlen, vlen)
    # Batch order: smallest staging first (fast PE ramp), then the largest
    # batch (its ACT-bound attention overlaps later batches' projections).
    active = [b for b in range(B) if qw[b] > 0]
    order = os.environ.get("BATCH_ORDER", "")
    if order:
        perm = [int(x) for x in order.split(",")]
        active = [b for b in perm if b in active]
    else:
        # Descending attention area: the long ACT-bound attention of big
        # batches overlaps later (PE-dense) projection phases, and the
        # kernel tail ends on a small, quickly-drained batch.
        active.sort(key=lambda b: -(kw[b] * qw[b]))

    nc = bass.Bass(
        "TRN2",
        target_bir_lowering=False,
        debug=False,
        enable_asserts=False,
        num_devices=NCORES,
    )

    qt_d, kt_d, vt_d, ot_d = {}, {}, {}, {}
    for b in active:
        qt_d[b] = nc.dram_tensor(f"qt{b}", [DM, qw[b]], f16,
                                 kind="ExternalInput").ap()
        kt_d[b] = nc.dram_tensor(f"kt{b}", [DM, kw[b]], f16,
                                 kind="ExternalInput").ap()
        vt_d[b] = nc.dram_tensor(f"vt{b}", [DM, kw[b]], f16,
                                 kind="ExternalInput").ap()
        ot_d[b] = nc.dram_tensor(f"ot{b}", [65, 2, qw[b]], f32,
                                 kind="ExternalOutput").ap()
    wq_d = nc.dram_tensor("wq", [DM, P], f16, kind="ExternalInput").ap()
    wk_d = nc.dram_tensor("wk", [DM, P], f16, kind="ExternalInput").ap()
    wv_d = nc.dram_tensor("wv", [DM, P], f16, kind="ExternalInput").ap()

    NCH = DM // P  # contraction chunks per projection

    with tile.TileContext(nc) as tc:
        with (
            tc.tile_pool(name="wpool", bufs=1) as wpool,
            tc.tile_pool(name="proj", bufs=3) as projpool,
            tc.tile_pool(name="stage", bufs=16) as stage,
            tc.tile_pool(name="sbig", bufs=6) as sbig,
            tc.tile_pool(name="outp", bufs=2) as outp,
            # PSUM budget (8 banks): pproj 2×1 + psc 2×2 + po0/po1 1×1 each
            tc.tile_pool(name="pproj", bufs=2, space="PSUM") as pproj,
            tc.tile_pool(name="psc", bufs=2, space="PSUM") as pscp,
            tc.tile_pool(name="ppo", bufs=1, space="PSUM") as ppo,
        ):
            # Weights for this core's head pair, DM on partitions: [128,8,128]
            # wq is loaded first; wk/wv are emitted after the first staging
            # DMA below so the first projection isn't queued behind them.
            w_sb = {}
            for name, ap in (("wq", wq_d), ("wk", wk_d), ("wv", wv_d)):
                w_sb[name] = wpool.tile([P, NCH, P], f16, tag=f"w_{name}",
                                        name=f"w_{name}")
            nc.sync.dma_start(w_sb["wq"],
                              wq_d.rearrange("(c p) m -> p c m", p=P))

            # Warm the PE HAM clock gate during the unavoidable initial DMA
            # wait: ~4us of dummy matmuls flips the PE from 1.2 to 2.4 GHz
            # before the first real projection arrives.
            if int(os.environ.get("PE_WARMUP", "1")):
                warm = wpool.tile([P, P], f16, tag="warm")
                nc.vector.memset(warm, 0.0)
                pwm = pproj.tile([P, 64], f32, tag="pproj")
                nwarm = 80
                for i in range(nwarm):
                    nc.tensor.matmul(pwm, lhsT=warm, rhs=warm[:, 0:64],
                                     start=(i == 0), stop=(i == nwarm - 1))
            first_stage = [True]

            def _stage(src_ap, w):
                st = stage.tile([P, NCH, w], f16, tag="stage")
                nc.sync.dma_start(st,
                                  src_ap.rearrange("(c p) n -> p c n", p=P))
                if first_stage[0]:
                    first_stage[0] = False
                    nc.sync.dma_start(
                        w_sb["wk"], wk_d.rearrange("(c p) m -> p c m", p=P))
                    nc.sync.dma_start(
                        w_sb["wv"], wv_d.rearrange("(c p) m -> p c m", p=P))
                return st

            for bi, b in enumerate(active):
                nkt = kw[b] // P

                qT = projpool.tile([P, qw[b]], f16, tag="qT")
                kT = projpool.tile([P, kw[b]], f16, tag="kT")
                # projected V + ones column per head: [kpos, head, ktile, 65]
                vnx = projpool.tile([P, 2, nkt, 65], f16, tag="vnx")

                # --- projections, q/k/v spans interleaved so the first
                # attention cells' inputs land as early as possible
                def _proj_q(off, w):
                    st = _stage(qt_d[b][:, off:off + w], w)
                    ps = pproj.tile([P, SPAN], f32, tag="pproj",
                                    name="ps_q")
                    for ch in range(NCH):
                        nc.tensor.matmul(ps[:, :w], lhsT=w_sb["wq"][:, ch],
                                         rhs=st[:, ch],
                                         start=(ch == 0),
                                         stop=(ch == NCH - 1))
                    nc.vector.tensor_copy(qT[:, off:off + w], ps[:, :w])

                def _proj_k(off, w):
                    st = _stage(kt_d[b][:, off:off + w], w)
                    ps = pproj.tile([P, SPAN], f32, tag="pproj",
                                    name="ps_k")
                    for ch in range(NCH):
                        nc.tensor.matmul(ps[:, :w], lhsT=w_sb["wk"][:, ch],
                                         rhs=st[:, ch],
                                         start=(ch == 0),
                                         stop=(ch == NCH - 1))
                    nc.vector.tensor_copy(kT[:, off:off + w], ps[:, :w])

                def _proj_v(off, w):
                    st = _stage(vt_d[b][:, off:off + w], w)
                    for kt in range(off // P, (off + w) // P):
                        o = kt * P - off
                        pv = pproj.tile([P, 2, 64], f32, tag="pproj",
                                        name="pv")
                        for ch in range(NCH):
                            nc.tensor.matmul(pv, lhsT=st[:, ch, o:o + P],
                                             rhs=w_sb["wv"][:, ch],
                                             start=(ch == 0),
                                             stop=(ch == NCH - 1))
                        nc.vector.tensor_copy(vnx[:, :, kt, 0:64], pv)

                steps = ([("q", s) for s in _splits(qw[b])] +
                         [("k", s) for s in _splits(kw[b])] +
                         [("v", s) for s in _splits(kw[b])])
                if os.environ.get("PROJ_INTERLEAVE", "1") == "1":
                    qs = [("q", s) for s in _splits(qw[b])]
                    ks = [("k", s) for s in _splits(kw[b])]
                    vs = [("v", s) for s in _splits(kw[b])]
                    steps = []
                    n = max(len(qs), len(ks), len(vs))
                    for i in range(n):
                        for lst in (qs, ks, vs):
                            if i < len(lst):
                                steps.append(lst[i])
                for kind, (off, w) in steps:
                    {"q": _proj_q, "k": _proj_k, "v": _proj_v}[kind](off, w)

                # ones columns (masked at the V_len boundary tile)
                nfull = vlen[b] // P
                if nfull > 0:
                    nc.vector.memset(vnx[:, :, 0:nfull, 64:65], 1.0)
                if nfull < nkt:  # partial boundary tile
                    r = vlen[b] - nfull * P
                    nc.vector.memset(vnx[:, :, nfull:nkt, 64:65], 0.0)
                    nc.vector.memset(vnx[0:r, :, nfull:nkt, 64:65], 1.0)

                # --- attention
                for off, w in _splits(qw[b]):
                    po0 = ppo.tile([65, SPAN], f32, tag="po0")
                    po1 = ppo.tile([65, SPAN], f32, tag="po1")
                    for kt in range(nkt):
                        ks = slice(kt * P, (kt + 1) * P)
                        psc = pscp.tile([P, 2, SPAN], f32, tag="psc")
                        nc.tensor.matmul(psc[:, 0, :w],
                                         lhsT=kT[0:64, ks],
                                         rhs=qT[0:64, off:off + w],
                                         start=True, stop=True,
                                         tile_position=(0, 0))
                        nc.tensor.matmul(psc[:, 1, :w],
                                         lhsT=kT[64:P, ks],
                                         rhs=qT[64:P, off:off + w],
                                         start=True, stop=True,
                                         tile_position=(64, 0))
                        ex = sbig.tile([P, 2, SPAN], f16, tag="exp")
                        nc.scalar.activation(ex[:, :, :w], psc[:, :, :w],
                                             AF.Exp)
                        first, last = (kt == 0), (kt == nkt - 1)
                        nc.tensor.matmul(po0[:, :w], lhsT=vnx[:, 0, kt],
                                         rhs=ex[:, 0, :w],
                                         start=first, stop=last)
                        nc.tensor.matmul(po1[:, :w], lhsT=vnx[:, 1, kt],
                                         rhs=ex[:, 1, :w],
                                         start=first, stop=last)
                    osb = outp.tile([65, 2, SPAN], f32, tag="osb")
                    nc.vector.tensor_copy(osb[:, 0, :w], po0[:, :w])
                    nc.vector.tensor_copy(osb[:, 1, :w], po1[:, :w])
                    nc.sync.dma_start(ot_d[b][:, :, off:off + w],
                                      osb[:, :, :w])

    _split_excess_waits(nc, mybir)
    return nc, qw, kw, active


def kernel(Q_seq, K_seq, V_seq, Q_len, V_len, WQ, WK, WV):
    global LAST_EXEC_NS, LAST_RESULTS, LAST_NC, LAST_IN_MAPS
    import concourse.bass_utils as bass_utils

    Q_seq = np.ascontiguousarray(np.asarray(Q_seq, dtype=np.float32))
    K_seq = np.ascontiguousarray(np.asarray(K_seq, dtype=np.float32))
    V_seq = np.ascontiguousarray(np.asarray(V_seq, dtype=np.float32))
    WQ = np.asarray(WQ, dtype=np.float32)
    WK = np.asarray(WK, dtype=np.float32)
    WV = np.asarray(WV, dtype=np.float32)
    qlen = [int(x) for x in np.asarray(Q_len).ravel()]
    vlen = [int(x) for x in np.asarray(V_len).ravel()]

    f16 = ml_dtypes.float16 if hasattr(ml_dtypes, "float16") else np.float16
    out = np.zeros((B, S, H * D), dtype=np.float32)

    # Degenerate batches (V_len==0): reference softmax of an all-masked row
    # is uniform over all S keys -> O row = mean of v rows.
    for b in range(B):
        if vlen[b] == 0 and qlen[b] > 0:
            v = V_seq[b] @ WV
            out[b, :qlen[b], :] = v.mean(axis=0, keepdims=True)

    key = (tuple(qlen), tuple(vlen))
    if key not in _PROGRAM_CACHE:
        _PROGRAM_CACHE[key] = _build_program(qlen, vlen)
    nc, qw, kw, active = _PROGRAM_CACHE[key]

    if active:
        WQs = (WQ / math.sqrt(D)).astype(f16)
        WKs = WK.astype(f16)
        WVs = WV.astype(f16)

        # Shared (core-independent) transposed activations, zero-padded.
        shared = {}
        for b in active:
            qt = np.zeros((DM, qw[b]), dtype=f16)
            qt[:, :qlen[b]] = Q_seq[b, :qlen[b], :].T
            kt = np.zeros((DM, kw[b]), dtype=f16)
            kt[:, :vlen[b]] = K_seq[b, :vlen[b], :].T
            vt = np.zeros((DM, kw[b]), dtype=f16)
            vt[:, :vlen[b]] = V_seq[b, :vlen[b], :].T
            shared[f"qt{b}"] = qt
            shared[f"kt{b}"] = kt
            shared[f"vt{b}"] = vt

        in_maps = []
        for c in range(NCORES):
            m = dict(shared)
            sl = slice(c * P, (c + 1) * P)
            m["wq"] = np.ascontiguousarray(WQs[:, sl])
            m["wk"] = np.ascontiguousarray(WKs[:, sl])
            m["wv"] = np.ascontiguousarray(WVs[:, sl])
            in_maps.append(m)

        trace = bool(int(os.environ.get("KERNEL_TRACE", "0")))
        try:
            res = bass_utils.run_bass_kernel_spmd(
                nc, in_maps, core_ids=list(range(NCORES)), trace=trace)
        except ModuleNotFoundError:
            # Profiling hook unavailable in this container; run untraced.
            os.environ["BASS_NEVER_TRACE"] = "1"
            res = bass_utils.run_bass_kernel_spmd(
                nc, in_maps, core_ids=list(range(NCORES)), trace=False)
        LAST_EXEC_NS = res.exec_time_ns
        LAST_RESULTS = res
        LAST_NC = nc
        LAST_IN_MAPS = in_maps

        for c in range(NCORES):
            r = res.results[c]
            for b in active:
                arr = r[f"ot{b}"]  # [65, 2, qw]: rows 0-63 O^T, row 64 den
                n = qlen[b]
                for h in (0, 1):
                    head = 2 * c + h
                    num = arr[0:64, h, :n]
                    den = arr[64, h, :n]
                    out[b, :n, head * 64:(head + 1) * 64] = \
                        (num / den[None, :]).T
    return out
